# revision 18
# baseline (speedup 1.0000x reference)
"""Causal single-head attention (B=4, S=4096, D=1024, H=64) on 8 TRN2 cores.

Sharding: 8 cores = 4 batches x 2 query-fold roles (role r owns 512-row
chunks {2s+r}). One SPMD program serves all 8 cores; role differences ride
in tiny input tensors (diag masks + rolesel).

Structure:
- inputs fp8e4 (or bf16) over DMA; weights bf16.
- projections col-tile two 512-blocks per pass (output partitions 0:64 and
  64:128 of one PSUM tile; two concurrent matmuls per contraction chunk).
- K^T pair-packed [128, 1024]: even 512-key groups on partitions 0:64, odd
  on 64:128, so score matmuls row-tile two K=64 blocks concurrently.
- exp per score-pair [128, 2, 512]: most slots on ScalarE (Exp, bias
  -ln 32), a tunable subset on VectorE via a Schraudolph int16 bit-trick
  (bf16 bits = 184.664*s + 15608.6). All paths yield exp(s)/32; the host
  num/den divide cancels the 1/32.
- denominator via an ones-column appended to V (PV matmul row 64).
- K/V fold-pair exchange via AllReduce(add): peer = sum - local.
"""

import numpy as np
import ml_dtypes

import concourse.bacc as bacc
import concourse.mybir as mybir
import concourse.tile as tile
from concourse.masks import make_identity
from concourse.bass_utils import run_bass_kernel_spmd

B, S, D, H = 4, 4096, 1024, 64
SBLK = 512
NCH = D // 128
QLOC = 2048

F32 = mybir.dt.float32
BF16 = mybir.dt.bfloat16
I16 = mybir.dt.int16
F8 = mybir.dt.float8e4

RG_PAIRS = [[0, 1], [2, 3], [4, 5], [6, 7]]

A16 = 128.0 / np.log(2.0)   # Schraudolph slope for bf16 bits
B16 = 15616.0 - 7.4         # 128*(127-5) - minimax c
LN32 = float(np.log(32.0))
VW = H + 1

CFG = dict(in8=False, ar=True,
           dve_slots=("cross2u0", "cross3u0"))


def build_prog(dve_slots):
    """Slot program. Each slot: two row-tiled 128-key score blocks
    (planes A/B), one exp, optional diag masks, two PV accumulations.
    kt: which K^T tile; j: 128-col index into it; qa/qb: q-tile per plane;
    po/va/m per plane; mask index or None. Epilogues appear as
    ("epi", s)."""
    prog = []

    def grp(name, kt, js, qa, qb, poa, vaa, ma_of, pob, vab, mb_of,
            mka=None, mkb=None):
        for j in js:
            prog.append(dict(
                name=name,
                kt=kt, j=j, qa=qa, qb=qb,
                poa=poa, vaa=vaa, ma=ma_of(j),
                pob=pob, vab=vab, mb=mb_of(j),
                mask_a=(mka(j) if mka else None),
                mask_b=(mkb(j) if mkb else None),
                dve=(name in dve_slots),
            ))

    J4 = range(4)
    grp("same1", "ktl", J4, 1, 1, 1, "vloc", lambda j: 2 * j,
        1, "vloc", lambda j: 2 * j + 1, mkb=lambda j: j)
    grp("same2u0", "ktl", J4, 2, 2, 2, "vloc", lambda j: 2 * j,
        2, "vloc", lambda j: 2 * j + 1)
    grp("mixed", "ktmxl", J4, 0, 2, 0, "vloc", lambda j: 2 * j,
        2, "vloc", lambda j: 8 + 2 * j, mka=lambda j: j, mkb=lambda j: j)
    grp("cross1", "ktp", J4, 1, 1, 1, "vpeer", lambda j: 2 * j,
        1, "vdd", lambda j: 2 * j + 1)
    prog.append(("epi", 1))
    grp("same3u0", "ktl", J4, 3, 3, 3, "vloc", lambda j: 2 * j,
        3, "vloc", lambda j: 2 * j + 1)
    grp("same3u1", "ktl", [4 + j for j in J4], 3, 3, 3, "vloc",
        lambda j: 2 * j, 3, "vloc", lambda j: 2 * j + 1,
        mkb=lambda j: j - 4)
    grp("cross2u0", "ktp", J4, 2, 2, 2, "vpeer", lambda j: 2 * j,
        2, "vpeer", lambda j: 2 * j + 1)
    grp("mixedx", "ktmxp", J4, 0, 2, 0, "vdd", lambda j: 2 * j,
        2, "vdd", lambda j: 8 + 2 * j)
    prog.append(("epi", 2))
    prog.append(("epi", 0))
    grp("cross3u0", "ktp", J4, 3, 3, 3, "vpeer", lambda j: 2 * j,
        3, "vpeer", lambda j: 2 * j + 1)
    grp("cross3u1", "ktp", [4 + j for j in J4], 3, 3, 3, "vpeer",
        lambda j: 2 * j, 3, "vdd", lambda j: 2 * j + 1)
    prog.append(("epi", 3))

    return prog


def prog_first_last(prog):
    writes = {}
    for idx, e in enumerate(prog):
        if isinstance(e, dict):
            writes.setdefault(e["poa"], []).append((idx, 0))
            writes.setdefault(e["pob"], []).append((idx, 1))
    first = {po: w[0] for po, w in writes.items()}
    last = {po: w[-1] for po, w in writes.items()}
    return first, last


def build_kernel(repeat: int = 1, in8=None, ar=None, dve_slots=None,
                 phase="full"):
    in8 = CFG["in8"] if in8 is None else in8
    ar = CFG["ar"] if ar is None else ar
    dve_slots = CFG["dve_slots"] if dve_slots is None else dve_slots
    IN_DT = F8 if in8 else BF16

    nc = bacc.Bacc("TRN2", target_bir_lowering=False, debug=False,
                   num_devices=8)

    qT = nc.dram_tensor("qT", [D, QLOC], IN_DT, kind="ExternalInput")
    kT = nc.dram_tensor("kT", [D, QLOC], IN_DT, kind="ExternalInput")
    vT = nc.dram_tensor("vT", [D, QLOC], BF16, kind="ExternalInput")
    wqT = nc.dram_tensor("wqT", [D, H], BF16, kind="ExternalInput")
    wkT = nc.dram_tensor("wkT", [D, H], BF16, kind="ExternalInput")
    wvT = nc.dram_tensor("wvT", [D, H], BF16, kind="ExternalInput")
    masks = nc.dram_tensor("masks", [4, 128, SBLK], BF16, kind="ExternalInput")
    rolesel = nc.dram_tensor("rolesel", [128, 2], F32, kind="ExternalInput")
    out = nc.dram_tensor("oT", [H + 1, QLOC], F32, kind="ExternalOutput")

    with tile.TileContext(nc) as tc:
        with (
            tc.tile_pool(name="const", bufs=1) as const_pool,
            tc.tile_pool(name="big", bufs=2) as big_pool,
            tc.tile_pool(name="strips", bufs=4) as strip_pool,
            tc.tile_pool(name="work", bufs=2) as work_pool,
            tc.tile_pool(name="pt", bufs=14) as pt_pool,
            tc.tile_pool(name="pp", bufs=1, space="PSUM") as pp,
            tc.tile_pool(name="ps_sc", bufs=2, space="PSUM") as ps_sc,
            tc.tile_pool(name="ps_o", bufs=3, space="PSUM") as ps_o,
            tc.tile_pool(name="dram", bufs=2, space="DRAM") as dram_pool,
        ):
            wq_sb = const_pool.tile([128, NCH, H], BF16)
            wk_sb = const_pool.tile([128, NCH, H], BF16)
            wv_sb = const_pool.tile([128, NCH, H], BF16)
            nc.sync.dma_start(wq_sb[:], wqT.rearrange("(c p) h -> p c h", p=128))
            nc.sync.dma_start(wk_sb[:], wkT.rearrange("(c p) h -> p c h", p=128))
            nc.sync.dma_start(wv_sb[:], wvT.rearrange("(c p) h -> p c h", p=128))
            mask_sb = const_pool.tile([128, 4, SBLK], BF16)
            nc.sync.dma_start(mask_sb[:], masks.rearrange("m p q -> p m q"))
            rs = const_pool.tile([128, 2], F32)
            nc.sync.dma_start(rs[:], rolesel[:])
            ident_f = const_pool.tile([128, 128], F32)
            make_identity(nc, ident_f[:])
            nln32 = const_pool.tile([128, 1], F32)
            nc.vector.memset(nln32[:], -LN32)

            for _rep in range(repeat):
                qt2 = big_pool.tile([128, QLOC], BF16, tag="qt2", name="qt2")
                kt2_loc = big_pool.tile([128, 1024], BF16, tag="ktl",
                                        name="kt2_loc")
                kt2_peer = big_pool.tile([128, 1024], BF16, tag="ktp",
                                         name="kt2_peer")
                ktmx_loc = big_pool.tile([128, SBLK], BF16, tag="kml",
                                         name="ktmx_loc")
                ktmx_peer = big_pool.tile([128, SBLK], BF16, tag="kmp",
                                          name="ktmx_peer")
                vloc = big_pool.tile([128, 16, VW], BF16, tag="vl",
                                     name="vloc")
                vpeer = big_pool.tile([128, 16, VW], BF16, tag="vp",
                                      name="vpeer")
                vpeer_dd = big_pool.tile([128, 16, VW], BF16, tag="vdd",
                                         name="vpeer_dd")
                ot_sb = big_pool.tile([H + 1, QLOC], F32, tag="ot",
                                      name="ot_sb")
                ksum = big_pool.tile([128, 1024], BF16, tag="ks",
                                     name="ksum")
                vsum = big_pool.tile([128, 16, H], BF16, tag="vs",
                                     name="vsum")
                nc.vector.memset(vloc[:, :, H], 1.0)

                def load_strip(src_dram, g, tag="xstrip"):
                    dt = BF16 if src_dram is vT else IN_DT
                    strip = strip_pool.tile([128, NCH, SBLK], dt, tag=tag)
                    nc.sync.dma_start(
                        strip[:],
                        src_dram[:, g * SBLK : (g + 1) * SBLK].rearrange(
                            "(c p) s -> p c s", p=128
                        ),
                    )
                    return strip

                def proj_pair(w_a, strip_a, w_b, strip_b):
                    pt = pp.tile([128, SBLK], F32, tag="proj")
                    for c in range(NCH):
                        nc.tensor.matmul(
                            pt[0:64, :], w_a[:, c, :], strip_a[:, c, :],
                            start=(c == 0), stop=(c == NCH - 1),
                        )
                        nc.tensor.matmul(
                            pt[64:128, :], w_b[:, c, :], strip_b[:, c, :],
                            start=(c == 0), stop=(c == NCH - 1),
                        )
                    return pt

                if phase == "dma":
                    for g in range(4):
                        for src in (kT, vT, qT):
                            strip = load_strip(src, g)
                            nc.vector.tensor_copy(
                                ot_sb[:1, :4], strip[:1, 0, :8].bitcast(BF16)
                                if in8 else strip[:1, 0, :4],
                            )
                    nc.vector.memset(ot_sb[:], 0.0)
                    nc.sync.dma_start(out[:], ot_sb[:])
                    continue

                # ---------- projections (emission order = DMA order) ------
                sk0 = load_strip(kT, 0)
                sk1 = load_strip(kT, 1)
                ppk = proj_pair(wk_sb, sk0, wk_sb, sk1)
                nc.vector.tensor_copy(kt2_loc[:, 0:SBLK], ppk[:])

                sq0 = load_strip(qT, 0)
                sq1 = load_strip(qT, 1)
                ppq = proj_pair(wq_sb, sq0, wq_sb, sq1)
                qst0 = work_pool.tile([128, SBLK], BF16, tag="qstage",
                                      bufs=3)
                nc.vector.tensor_copy(qst0[:], ppq[:])
                nc.sync.dma_start(qt2[0:64, 0:512], qst0[0:64, :])
                nc.sync.dma_start(qt2[0:64, 512:1024], qst0[64:128, :])
                nc.sync.dma_start(qt2[64:128, 0:512], qst0[0:64, :])
                nc.sync.dma_start(qt2[64:128, 512:1024], qst0[64:128, :])

                sk2 = load_strip(kT, 2)
                sq2 = load_strip(qT, 2)
                ppa = proj_pair(wk_sb, sk2, wq_sb, sq2)
                nc.vector.tensor_copy(kt2_loc[0:64, SBLK:1024], ppa[0:64, :])
                qst2 = work_pool.tile([128, SBLK], BF16, tag="qstage",
                                      bufs=3)
                nc.vector.tensor_copy(qst2[64:128, :], ppa[64:128, :])
                nc.sync.dma_start(qt2[0:64, 1024:1536], qst2[64:128, :])
                nc.sync.dma_start(qt2[64:128, 1024:1536], qst2[64:128, :])

                sk3 = load_strip(kT, 3)
                sq3 = load_strip(qT, 3)
                ppb = proj_pair(wk_sb, sk3, wq_sb, sq3)
                nc.vector.tensor_copy(kt2_loc[64:128, SBLK:1024], ppb[0:64, :])
                qst3 = work_pool.tile([128, SBLK], BF16, tag="qstage",
                                      bufs=3)
                nc.vector.tensor_copy(qst3[64:128, :], ppb[64:128, :])
                nc.sync.dma_start(qt2[0:64, 1536:2048], qst3[64:128, :])
                nc.sync.dma_start(qt2[64:128, 1536:2048], qst3[64:128, :])

                nc.sync.dma_start(ktmx_loc[0:64, :], kt2_loc[0:64, 0:SBLK])
                nc.sync.dma_start(ktmx_loc[64:128, :], kt2_loc[0:64, SBLK:1024])

                k_in = dram_pool.tile([128, 1024], BF16, tag="k_in")
                if phase == "full":
                    nc.gpsimd.dma_start(k_in[:], kt2_loc[:])
                if phase == "full" and ar:
                    k_out = dram_pool.tile([128, 1024], BF16, tag="k_out")
                    nc.gpsimd.collective_compute(
                        "AllReduce", mybir.AluOpType.add,
                        replica_groups=RG_PAIRS,
                        ins=[k_in.opt()], outs=[k_out.opt()],
                    )
                elif phase == "full":
                    k_out = dram_pool.tile([2, 128, 1024], BF16, tag="k_out")
                    nc.gpsimd.collective_compute(
                        "AllGather", mybir.AluOpType.bypass,
                        replica_groups=RG_PAIRS,
                        ins=[k_in.opt()], outs=[k_out.opt()],
                    )

                # ---------- attention program ----------
                prog = build_prog(dve_slots)
                if phase == "same":
                    keep = ("same1", "same2u0", "mixed", "same3u0",
                            "same3u1")
                    prog = [e for e in prog if isinstance(e, dict)
                            and e["name"] in keep]
                    prog += [("epi", s) for s in (1, 2, 0, 3)]
                elif phase == "proj":
                    prog = []
                first, last = prog_first_last(prog)
                tiles = {"ktl": kt2_loc, "ktp": kt2_peer,
                         "ktmxl": ktmx_loc, "ktmxp": ktmx_peer,
                         "vloc": vloc, "vpeer": vpeer, "vdd": vpeer_dd}
                pos = {}

                def get_po(s):
                    if s not in pos:
                        pos[s] = ps_o.tile([H + 1, SBLK], F32, tag="oT", name=f"po{s}")
                    return pos[s]

                def emit_scores_exp(e):
                    kt = tiles[e["kt"]]
                    j = e["j"]
                    ps2 = ps_sc.tile([128, 2, SBLK], F32, tag="scores")
                    nc.tensor.matmul(
                        ps2[:, 0, :], kt[0:64, 128 * j : 128 * (j + 1)],
                        qt2[0:64, e["qa"] * SBLK : (e["qa"] + 1) * SBLK],
                        start=True, stop=True,
                    )
                    nc.tensor.matmul(
                        ps2[:, 1, :], kt[64:128, 128 * j : 128 * (j + 1)],
                        qt2[64:128, e["qb"] * SBLK : (e["qb"] + 1) * SBLK],
                        start=True, stop=True,
                    )
                    if e["dve"]:
                        pti = pt_pool.tile([128, 2, SBLK], I16, tag="pt")
                        nc.vector.tensor_scalar(
                            pti[:], ps2[:], A16, B16,
                            mybir.AluOpType.mult, mybir.AluOpType.add,
                        )
                        def plane(jj):
                            return pti[:, jj, :].bitcast(BF16)
                    else:
                        ptt = pt_pool.tile([128, 2, SBLK], BF16, tag="pt")
                        nc.scalar.activation(
                            ptt[:], ps2[:],
                            mybir.ActivationFunctionType.Exp, bias=nln32[:],
                        )
                        def plane(jj):
                            return ptt[:, jj, :]
                    if e["mask_a"] is not None:
                        nc.vector.tensor_mul(
                            plane(0), plane(0), mask_sb[:, e["mask_a"], :]
                        )
                    if e["mask_b"] is not None:
                        nc.vector.tensor_mul(
                            plane(1), plane(1), mask_sb[:, e["mask_b"], :]
                        )
                    return plane

                def emit_pv(idx, e, plane):
                    nc.tensor.matmul(
                        get_po(e["poa"])[:, :],
                        tiles[e["vaa"]][:, e["ma"], :VW], plane(0),
                        start=(first[e["poa"]] == (idx, 0)),
                        stop=(last[e["poa"]] == (idx, 0)),
                    )
                    nc.tensor.matmul(
                        get_po(e["pob"])[:, :],
                        tiles[e["vab"]][:, e["mb"], :VW], plane(1),
                        start=(first[e["pob"]] == (idx, 1)),
                        stop=(last[e["pob"]] == (idx, 1)),
                    )

                def emit_epi(s):
                    po = pos.pop(s)
                    nc.vector.tensor_copy(
                        ot_sb[:, s * SBLK : (s + 1) * SBLK], po[:]
                    )
                    nc.sync.dma_start(
                        out[:, s * SBLK : (s + 1) * SBLK],
                        ot_sb[:, s * SBLK : (s + 1) * SBLK],
                    )

                DEFER_N = 8 if phase in ("full", "same") else 0
                deferred = []
                if phase in ("full", "same"):
                    for idx in range(DEFER_N):
                        e = prog[idx]
                        deferred.append((idx, e, emit_scores_exp(e)))

                def v_pair(u):
                    sva = load_strip(vT, 2 * u)
                    svb = load_strip(vT, 2 * u + 1)
                    ppv = proj_pair(wv_sb, sva, wv_sb, svb)
                    vstage = work_pool.tile([128, SBLK], F32, tag="vstage")
                    nc.vector.tensor_copy(vstage[:], ppv[:])
                    tp = pp.tile([128, 8, H], F32, tag="proj", name="tp")
                    for i in range(4):
                        nc.tensor.transpose(
                            tp[:, 2 * i, :],
                            vstage[0:64, i * 128 : (i + 1) * 128],
                            ident_f[0:64, 0:64],
                        )
                        nc.tensor.transpose(
                            tp[:, 2 * i + 1, :],
                            vstage[64:128, i * 128 : (i + 1) * 128],
                            ident_f[64:128, 64:128],
                        )
                    nc.vector.tensor_copy(vloc[:, 8 * u : 8 * u + 8, :H], tp[:])

                v_pair(0)
                v_pair(1)

                for idx, e, plane in deferred:
                    emit_pv(idx, e, plane)
                deferred = []

                v_in = dram_pool.tile([128, 16, H], BF16, tag="v_in")
                if phase == "full":
                    nc.gpsimd.dma_start(v_in[:], vloc[:, :, :H])
                if phase == "full" and ar:
                    v_out = dram_pool.tile([128, 16, H], BF16, tag="v_out")
                    nc.gpsimd.collective_compute(
                        "AllReduce", mybir.AluOpType.add,
                        replica_groups=RG_PAIRS,
                        ins=[v_in.opt()], outs=[v_out.opt()],
                    )
                elif phase == "full":
                    v_out = dram_pool.tile([2, 128, 16, H], BF16, tag="v_out")
                    nc.gpsimd.collective_compute(
                        "AllGather", mybir.AluOpType.bypass,
                        replica_groups=RG_PAIRS,
                        ins=[v_in.opt()], outs=[v_out.opt()],
                    )

                # ---------- peer combine ----------
                if phase == "full" and ar:
                    nc.sync.dma_start(ksum[:], k_out[:])
                    nc.vector.tensor_sub(kt2_peer[:], ksum[:], kt2_loc[:])
                    nc.sync.dma_start(vsum[:], v_out[:])
                    nc.vector.tensor_sub(
                        vpeer[:, :, :H], vsum[:], vloc[:, :, :H]
                    )
                elif phase == "full":
                    k0s = work_pool.tile([128, 1024], BF16, tag="k0s")
                    nc.sync.dma_start(k0s[:], k_out[0])
                    nc.sync.dma_start(ksum[:], k_out[1])
                    ktmp = work_pool.tile([128, 1024], BF16, tag="ktmp")
                    nc.vector.tensor_scalar_mul(ktmp[:], k0s[:], rs[:, 0:1])
                    nc.vector.tensor_scalar_mul(kt2_peer[:], ksum[:], rs[:, 1:2])
                    nc.vector.tensor_add(kt2_peer[:], kt2_peer[:], ktmp[:])
                    v0s = work_pool.tile([128, 16, H], BF16, tag="v0s")
                    nc.sync.dma_start(v0s[:], v_out[0])
                    nc.sync.dma_start(vsum[:], v_out[1])
                    vtmp = work_pool.tile([128, 16, H], BF16, tag="vtmp")
                    nc.vector.tensor_scalar_mul(vtmp[:], v0s[:], rs[:, 0:1])
                    nc.vector.tensor_scalar_mul(
                        vpeer[:, :, :H], vsum[:], rs[:, 1:2]
                    )
                    nc.vector.tensor_add(
                        vpeer[:, :, :H], vpeer[:, :, :H], vtmp[:]
                    )
                if phase != "full":
                    pass
                else:
                    nc.vector.memset(vpeer[:, :, H], 1.0)
                if phase == "full":
                    nc.vector.tensor_scalar_mul(
                        vpeer_dd[:], vpeer[:], rs[:, 0:1]
                    )
                    nc.sync.dma_start(
                        ktmx_peer[0:64, :], kt2_peer[0:64, 0:SBLK]
                    )
                    nc.sync.dma_start(
                        ktmx_peer[64:128, :], kt2_peer[0:64, SBLK:1024]
                    )

                # ---------- remaining attention slots ----------
                for idx in range(DEFER_N, len(prog)):
                    e = prog[idx]
                    if not isinstance(e, dict):
                        emit_epi(e[1])
                        continue
                    emit_pv(idx, e, emit_scores_exp(e))
                if phase not in ("full", "same"):
                    nc.vector.memset(ot_sb[:], 0.0)
                    nc.sync.dma_start(out[:], ot_sb[:])

    nc.compile()
    return nc


def fold_rows(r):
    return np.concatenate(
        [np.arange(512 * (2 * s + r), 512 * (2 * s + r) + 512)
         for s in range(4)]
    )


def make_in_maps(q, k, v, Wq, Wk, Wv, in8=None):
    in8 = CFG["in8"] if in8 is None else in8
    in_np = ml_dtypes.float8_e4m3 if in8 else ml_dtypes.bfloat16
    scale = 1.0 / np.sqrt(np.float32(H))
    wqT = np.ascontiguousarray((Wq * scale).T).astype(ml_dtypes.bfloat16)
    wkT = np.ascontiguousarray(Wk.T).astype(ml_dtypes.bfloat16)
    wvT = np.ascontiguousarray(Wv.T).astype(ml_dtypes.bfloat16)

    kk = np.arange(128)[:, None]
    qq = np.arange(SBLK)[None, :]
    msk = np.stack(
        [(qq >= kk + 128 * m) for m in range(4)]
    ).astype(ml_dtypes.bfloat16)

    in_maps = []
    for c in range(8):
        b, r = c // 2, c % 2
        rows = fold_rows(r)
        rsel = np.zeros((128, 2), dtype=np.float32)
        rsel[:, 0] = 1.0 if r == 1 else 0.0
        rsel[:, 1] = 1.0 if r == 0 else 0.0
        in_maps.append(
            {
                "qT": np.ascontiguousarray(q[b][rows].T).astype(in_np),
                "kT": np.ascontiguousarray(k[b][rows].T).astype(in_np),
                "vT": np.ascontiguousarray(v[b][rows].T).astype(
                    ml_dtypes.bfloat16),
                "wqT": wqT,
                "wkT": wkT,
                "wvT": wvT,
                "masks": msk,
                "rolesel": rsel,
            }
        )
    return in_maps


def assemble_output(results):
    out = np.zeros((B, S, H), dtype=np.float32)
    for c in range(8):
        b, r = c // 2, c % 2
        oT = results[c]["oT"]
        for s in range(4):
            num = oT[:H, s * SBLK : (s + 1) * SBLK]
            den = oT[H, s * SBLK : (s + 1) * SBLK]
            g = 512 * (2 * s + r)
            out[b, g : g + 512, :] = (num / den[None, :]).T
    return out


_NC_CACHE = {}


def kernel(q, k, v, Wq, Wk, Wv):
    q = np.asarray(q, dtype=np.float32)
    k = np.asarray(k, dtype=np.float32)
    v = np.asarray(v, dtype=np.float32)
    Wq = np.asarray(Wq, dtype=np.float32)
    Wk = np.asarray(Wk, dtype=np.float32)
    Wv = np.asarray(Wv, dtype=np.float32)

    if "nc" not in _NC_CACHE:
        _NC_CACHE["nc"] = build_kernel(phase=CFG.get("phase", "full"))
    nc = _NC_CACHE["nc"]
    in_maps = make_in_maps(q, k, v, Wq, Wk, Wv)
    import os
    last_exc = None
    for attempt in range(int(os.environ.get('KER_RETRIES', '3'))):
        try:
            res = run_bass_kernel_spmd(nc, in_maps, core_ids=list(range(8)))
            return assemble_output(res.results)
        except Exception as e:
            last_exc = e
            import time as _time

            _time.sleep(15 * (attempt + 1))
    raise last_exc
